# revision 3
# baseline (speedup 1.0000x reference)
"""AFNO1D mixing kernel for 8 Trainium2 NeuronCores.

Strategy: tensor-parallel over the 8 block-diagonal channel blocks
(NUM_BLOCKS=8 == n_cores), one block of 96 channels per core, all 4
batch samples on every core. The rfft/irfft are factored into two-step
Cooley-Tukey real matmul stages (S=8192 = 64*128, modes k = 64e+d),
so the whole pipeline is real einsums + elementwise ops -- no FFT op,
which the Neuron compiler cannot ingest. Math validated to 3e-8 vs the
jax reference in float64 prototype.

  t = 128a + b:  G[d,b]   = sum_a cos/sin(2pi a d/64) x[128a+b] /sqrt(S)
  k = 64e + d:   X[64e+d] = sum_b w8192^{b(64e+d)} G[d,b]
  per-mode complex block MLP (relu, softshrink)
  inverse: A'[d,v] = sum_e ctil_k w8192^{+v k} O2[64e+d];
           y[128u+v] = Re sum_d w64^{+ud} A'[d,v];  y += x
"""
import numpy as np

HIDDEN = 768
NUM_BLOCKS = 8
CH = HIDDEN // NUM_BLOCKS  # 96
LAMBDA = 0.01
S, A_, B_, D_, E_ = 8192, 64, 128, 64, 65

_cache = {}


def _build_consts():
    a = np.arange(A_)[:, None]
    d = np.arange(D_)[None, :]
    ang = 2 * np.pi * a * d / 64.0
    st1c = (np.cos(ang) / np.sqrt(S)).astype(np.float32)   # [a, d]
    st1s = (-np.sin(ang) / np.sqrt(S)).astype(np.float32)  # [a, d]

    b = np.arange(B_)[:, None]
    e = np.arange(E_)[None, :]
    # M2[d]: [b, e] cos/sin of 2pi b (64e+d)/S
    ang2 = 2 * np.pi * b[None] * (64 * e[None] + np.arange(D_)[:, None, None]) / S
    M2r = np.cos(ang2).astype(np.float32)   # [d, b, e]
    M2i = (-np.sin(ang2)).astype(np.float32)
    M2r[1:, :, 64] = 0.0
    M2i[1:, :, 64] = 0.0

    ee = np.arange(E_)[:, None]
    v = np.arange(B_)[None, :]
    k3 = 64 * ee[None] + np.arange(D_)[:, None, None]       # [d, e, 1]
    ctil = np.where((k3 == 0) | (k3 == 4096), 1.0, 2.0) / np.sqrt(S)
    ang3 = 2 * np.pi * v[None] * k3 / S
    C3r = (ctil * np.cos(ang3)).astype(np.float32)          # [d, e, v]
    C3i = (ctil * np.sin(ang3)).astype(np.float32)
    C3r[1:, 64, :] = 0.0
    C3i[1:, 64, :] = 0.0

    u = np.arange(A_)[None, :]
    dc = np.arange(D_)[:, None]
    ST3c = np.cos(2 * np.pi * dc * u / 64.0).astype(np.float32)   # [d, u]
    ST3s = (-np.sin(2 * np.pi * dc * u / 64.0)).astype(np.float32)
    return st1c, st1s, M2r, M2i, C3r, C3i, ST3c, ST3s


def _make_block_fn(jnp):
    st1c, st1s, M2r, M2i, C3r, C3i, ST3c, ST3s = [
        jnp.asarray(c) for c in _build_consts()]

    def block_fn(xs, w1r, w1i, b1r, b1i, w2r, w2i, b2r, b2i):
        # xs: [4, 8192, 96] one channel block, all batches
        NB4 = xs.shape[0]
        xv = xs.reshape(NB4, A_, B_, CH)
        # stage 1 (contract a): G [n, d, b, c]
        Gr = jnp.einsum('ad,nabc->ndbc', st1c, xv)
        Gi = jnp.einsum('ad,nabc->ndbc', st1s, xv)
        # stage 2 (contract b): X [n, d, e, c]
        Xr = jnp.einsum('dbe,ndbc->ndec', M2r, Gr) - jnp.einsum('dbe,ndbc->ndec', M2i, Gi)
        Xi = jnp.einsum('dbe,ndbc->ndec', M2r, Gi) + jnp.einsum('dbe,ndbc->ndec', M2i, Gr)
        # MLP layer 1 (contract c)
        Hr = jnp.maximum(jnp.einsum('ndec,cj->ndej', Xr, w1r)
                         - jnp.einsum('ndec,cj->ndej', Xi, w1i) + b1r, 0.0)
        Hi = jnp.maximum(jnp.einsum('ndec,cj->ndej', Xi, w1r)
                         + jnp.einsum('ndec,cj->ndej', Xr, w1i) + b1i, 0.0)
        # MLP layer 2
        Or = (jnp.einsum('ndej,jc->ndec', Hr, w2r)
              - jnp.einsum('ndej,jc->ndec', Hi, w2i) + b2r)
        Oi = (jnp.einsum('ndej,jc->ndec', Hr, w2i)
              + jnp.einsum('ndej,jc->ndec', Hi, w2r) + b2i)
        Or = Or - jnp.clip(Or, -LAMBDA, LAMBDA)
        Oi = Oi - jnp.clip(Oi, -LAMBDA, LAMBDA)
        # inverse stage 1 (contract e): A' [n, d, v, c]
        Ar = jnp.einsum('dev,ndec->ndvc', C3r, Or) - jnp.einsum('dev,ndec->ndvc', C3i, Oi)
        Ai = jnp.einsum('dev,ndec->ndvc', C3i, Or) + jnp.einsum('dev,ndec->ndvc', C3r, Oi)
        # inverse stage 2 (contract d): y [n, u, v, c]
        y = jnp.einsum('du,ndvc->nuvc', ST3c, Ar) + jnp.einsum('du,ndvc->nuvc', ST3s, Ai)
        return y.reshape(NB4, S, CH) + xs

    return block_fn


def _get_impl():
    if 'impl' in _cache:
        return _cache['impl']
    impl = None
    try:
        import jax
        import jax.numpy as jnp
        devs = jax.devices()
        block_fn = _make_block_fn(jnp)
        if len(devs) >= NUM_BLOCKS:
            pfn = jax.pmap(block_fn, devices=devs[:NUM_BLOCKS])
            impl = ('pmap', pfn)
        else:
            impl = ('jit', jax.jit(jax.vmap(block_fn)))
    except Exception:
        impl = ('numpy', None)
    _cache['impl'] = impl
    return impl


def _numpy_fallback(x, w1, b1, w2, b2):
    xf = x.astype(np.float32)
    B = x.shape[0]
    X = np.fft.rfft(xf, axis=1, norm="ortho")
    X = X.reshape(B, -1, NUM_BLOCKS, CH)
    xr, xi = X.real.astype(np.float32), X.imag.astype(np.float32)
    mul = lambda inp, w: np.einsum("bmnd,ndk->bmnk", inp, w, optimize=True)
    o1r = np.maximum(mul(xr, w1[0]) - mul(xi, w1[1]) + b1[0], 0.0)
    o1i = np.maximum(mul(xi, w1[0]) + mul(xr, w1[1]) + b1[1], 0.0)
    o2r = mul(o1r, w2[0]) - mul(o1i, w2[1]) + b2[0]
    o2i = mul(o1r, w2[1]) + mul(o1i, w2[0]) + b2[1]
    o2r -= np.clip(o2r, -LAMBDA, LAMBDA)
    o2i -= np.clip(o2i, -LAMBDA, LAMBDA)
    Y = (o2r + 1j * o2i).reshape(B, -1, HIDDEN)
    y = np.fft.irfft(Y, n=S, axis=1, norm="ortho").astype(x.dtype)
    return y + x


def kernel(x, w1, b1, w2, b2):
    x = np.ascontiguousarray(np.asarray(x, dtype=np.float32))
    w1 = np.asarray(w1, np.float32); b1 = np.asarray(b1, np.float32)
    w2 = np.asarray(w2, np.float32); b2 = np.asarray(b2, np.float32)
    kind, fn = _get_impl()
    if kind == 'numpy':
        return _numpy_fallback(x, w1, b1, w2, b2)
    try:
        import time as _time
        B = x.shape[0]
        # [8, 4, 8192, 96] one channel block per device
        xsh = np.ascontiguousarray(
            x.reshape(B, S, NUM_BLOCKS, CH).transpose(2, 0, 1, 3))
        args = (xsh, w1[0], w1[1], b1[0], b1[1], w2[0], w2[1], b2[0], b2[1])
        t0 = _time.time()
        out = fn(*args)
        out = np.asarray(out)  # [8, 4, 8192, 96]
        globals()['_last_exec_ns'] = (_time.time() - t0) * 1e9
        return np.ascontiguousarray(
            out.transpose(1, 2, 0, 3).reshape(B, S, HIDDEN)).astype(np.float32)
    except Exception:
        _cache['impl'] = ('numpy', None)
        return _numpy_fallback(x, w1, b1, w2, b2)


if __name__ == "__main__":
    import time
    rng = np.random.default_rng(0)
    x = rng.standard_normal((4, S, HIDDEN), dtype=np.float32)
    w1 = 0.02 * rng.standard_normal((2, NUM_BLOCKS, CH, CH)).astype(np.float32)
    b1 = 0.02 * rng.standard_normal((2, NUM_BLOCKS, CH)).astype(np.float32)
    w2 = 0.02 * rng.standard_normal((2, NUM_BLOCKS, CH, CH)).astype(np.float32)
    b2 = 0.02 * rng.standard_normal((2, NUM_BLOCKS, CH)).astype(np.float32)
    y1 = kernel(x, w1, b1, w2, b2)
    t0 = time.time(); y1 = kernel(x, w1, b1, w2, b2); t1 = time.time()
    y2 = _numpy_fallback(x, w1, b1, w2, b2)
    print("self-check rel err:",
          np.linalg.norm(y1 - y2) / np.linalg.norm(y2),
          "steady wall:", (t1 - t0) * 1e3, "ms")


# revision 6
# speedup vs baseline: 1.9884x; 1.9884x over previous
"""AFNO1D mixing kernel for 8 Trainium2 NeuronCores.

Strategy: tensor-parallel over the 8 block-diagonal channel blocks
(NUM_BLOCKS=8 == n_cores), one block of 96 channels per core, all 4
batch samples on every core. The rfft/irfft are factored into two-step
Cooley-Tukey real matmul stages (S=8192 = 64*128, modes k = 64e+d),
so the whole pipeline is real einsums + elementwise ops -- no FFT op,
which the Neuron compiler cannot ingest. Math validated to 3e-8 vs the
jax reference in float64 prototype.

  t = 128a + b:  G[d,b]   = sum_a cos/sin(2pi a d/64) x[128a+b] /sqrt(S)
  k = 64e + d:   X[64e+d] = sum_b w8192^{b(64e+d)} G[d,b]
  per-mode complex block MLP (relu, softshrink)
  inverse: A'[d,v] = sum_e ctil_k w8192^{+v k} O2[64e+d];
           y[128u+v] = Re sum_d w64^{+ud} A'[d,v];  y += x
"""
import numpy as np

HIDDEN = 768
NUM_BLOCKS = 8
CH = HIDDEN // NUM_BLOCKS  # 96
LAMBDA = 0.01
S, A_, B_, D_, E_ = 8192, 64, 128, 64, 65

_cache = {}


def _build_consts():
    a = np.arange(A_)[:, None]
    d = np.arange(D_)[None, :]
    ang = 2 * np.pi * a * d / 64.0
    st1c = (np.cos(ang) / np.sqrt(S)).astype(np.float32)   # [a, d]
    st1s = (-np.sin(ang) / np.sqrt(S)).astype(np.float32)  # [a, d]

    b = np.arange(B_)[:, None]
    e = np.arange(E_)[None, :]
    # M2[d]: [b, e] cos/sin of 2pi b (64e+d)/S
    ang2 = 2 * np.pi * b[None] * (64 * e[None] + np.arange(D_)[:, None, None]) / S
    M2r = np.cos(ang2).astype(np.float32)   # [d, b, e]
    M2i = (-np.sin(ang2)).astype(np.float32)
    M2r[1:, :, 64] = 0.0
    M2i[1:, :, 64] = 0.0

    ee = np.arange(E_)[:, None]
    v = np.arange(B_)[None, :]
    k3 = 64 * ee[None] + np.arange(D_)[:, None, None]       # [d, e, 1]
    ctil = np.where((k3 == 0) | (k3 == 4096), 1.0, 2.0) / np.sqrt(S)
    ang3 = 2 * np.pi * v[None] * k3 / S
    C3r = (ctil * np.cos(ang3)).astype(np.float32)          # [d, e, v]
    C3i = (ctil * np.sin(ang3)).astype(np.float32)
    C3r[1:, 64, :] = 0.0
    C3i[1:, 64, :] = 0.0

    u = np.arange(A_)[None, :]
    dc = np.arange(D_)[:, None]
    ST3c = np.cos(2 * np.pi * dc * u / 64.0).astype(np.float32)   # [d, u]
    ST3s = (-np.sin(2 * np.pi * dc * u / 64.0)).astype(np.float32)
    return st1c, st1s, M2r, M2i, C3r, C3i, ST3c, ST3s


def _make_block_fn(jnp):
    st1c, st1s, M2r, M2i, C3r, C3i, ST3c, ST3s = [
        jnp.asarray(c) for c in _build_consts()]

    def block_fn(xs, w1r, w1i, b1r, b1i, w2r, w2i, b2r, b2i):
        # xs: [4, 8192, 96] one channel block, all batches (bf16 on the wire)
        NB4 = xs.shape[0]
        xs = xs.astype(jnp.float32)
        xv = xs.reshape(NB4, A_, B_, CH)
        # stage 1 (contract a): G [n, d, b, c]
        Gr = jnp.einsum('ad,nabc->ndbc', st1c, xv)
        Gi = jnp.einsum('ad,nabc->ndbc', st1s, xv)
        # stage 2 (contract b): X [n, d, e, c]
        Xr = jnp.einsum('dbe,ndbc->ndec', M2r, Gr) - jnp.einsum('dbe,ndbc->ndec', M2i, Gi)
        Xi = jnp.einsum('dbe,ndbc->ndec', M2r, Gi) + jnp.einsum('dbe,ndbc->ndec', M2i, Gr)
        # MLP layer 1 (contract c)
        Hr = jnp.maximum(jnp.einsum('ndec,cj->ndej', Xr, w1r)
                         - jnp.einsum('ndec,cj->ndej', Xi, w1i) + b1r, 0.0)
        Hi = jnp.maximum(jnp.einsum('ndec,cj->ndej', Xi, w1r)
                         + jnp.einsum('ndec,cj->ndej', Xr, w1i) + b1i, 0.0)
        # MLP layer 2
        Or = (jnp.einsum('ndej,jc->ndec', Hr, w2r)
              - jnp.einsum('ndej,jc->ndec', Hi, w2i) + b2r)
        Oi = (jnp.einsum('ndej,jc->ndec', Hr, w2i)
              + jnp.einsum('ndej,jc->ndec', Hi, w2r) + b2i)
        Or = Or - jnp.clip(Or, -LAMBDA, LAMBDA)
        Oi = Oi - jnp.clip(Oi, -LAMBDA, LAMBDA)
        # inverse stage 1 (contract e): A' [n, d, v, c]
        Ar = jnp.einsum('dev,ndec->ndvc', C3r, Or) - jnp.einsum('dev,ndec->ndvc', C3i, Oi)
        Ai = jnp.einsum('dev,ndec->ndvc', C3i, Or) + jnp.einsum('dev,ndec->ndvc', C3r, Oi)
        # inverse stage 2 (contract d): y [n, u, v, c]
        y = jnp.einsum('du,ndvc->nuvc', ST3c, Ar) + jnp.einsum('du,ndvc->nuvc', ST3s, Ai)
        return (y.reshape(NB4, S, CH) + xs).astype(jnp.bfloat16)

    return block_fn


def _get_impl():
    if 'impl' in _cache:
        return _cache['impl']
    impl = None
    try:
        import jax
        import jax.numpy as jnp
        devs = jax.devices()
        block_fn = _make_block_fn(jnp)
        if len(devs) >= NUM_BLOCKS:
            pfn = jax.pmap(block_fn, devices=devs[:NUM_BLOCKS])
            impl = ('pmap', pfn)
        else:
            impl = ('jit', jax.jit(jax.vmap(block_fn)))
    except Exception:
        impl = ('numpy', None)
    _cache['impl'] = impl
    return impl


def _numpy_fallback(x, w1, b1, w2, b2):
    xf = x.astype(np.float32)
    B = x.shape[0]
    X = np.fft.rfft(xf, axis=1, norm="ortho")
    X = X.reshape(B, -1, NUM_BLOCKS, CH)
    xr, xi = X.real.astype(np.float32), X.imag.astype(np.float32)
    mul = lambda inp, w: np.einsum("bmnd,ndk->bmnk", inp, w, optimize=True)
    o1r = np.maximum(mul(xr, w1[0]) - mul(xi, w1[1]) + b1[0], 0.0)
    o1i = np.maximum(mul(xi, w1[0]) + mul(xr, w1[1]) + b1[1], 0.0)
    o2r = mul(o1r, w2[0]) - mul(o1i, w2[1]) + b2[0]
    o2i = mul(o1r, w2[1]) + mul(o1i, w2[0]) + b2[1]
    o2r -= np.clip(o2r, -LAMBDA, LAMBDA)
    o2i -= np.clip(o2i, -LAMBDA, LAMBDA)
    Y = (o2r + 1j * o2i).reshape(B, -1, HIDDEN)
    y = np.fft.irfft(Y, n=S, axis=1, norm="ortho").astype(x.dtype)
    return y + x


def kernel(x, w1, b1, w2, b2):
    x = np.ascontiguousarray(np.asarray(x, dtype=np.float32))
    w1 = np.asarray(w1, np.float32); b1 = np.asarray(b1, np.float32)
    w2 = np.asarray(w2, np.float32); b2 = np.asarray(b2, np.float32)
    kind, fn = _get_impl()
    if kind == 'numpy':
        return _numpy_fallback(x, w1, b1, w2, b2)
    try:
        import time as _time
        import ml_dtypes
        B = x.shape[0]
        # [8, 4, 8192, 96] one channel block per device; bf16 on the wire
        # (residual path stays well within the 2e-2 tolerance)
        xsh = np.ascontiguousarray(
            x.reshape(B, S, NUM_BLOCKS, CH).transpose(2, 0, 1, 3)
        ).astype(ml_dtypes.bfloat16)
        args = (xsh, w1[0], w1[1], b1[0], b1[1], w2[0], w2[1], b2[0], b2[1])
        t0 = _time.time()
        out = fn(*args)
        out = np.asarray(out)  # [8, 4, 8192, 96] bf16
        globals()['_last_exec_ns'] = (_time.time() - t0) * 1e9
        out = out.astype(np.float32)
        return np.ascontiguousarray(
            out.transpose(1, 2, 0, 3).reshape(B, S, HIDDEN)).astype(np.float32)
    except Exception:
        _cache['impl'] = ('numpy', None)
        return _numpy_fallback(x, w1, b1, w2, b2)


if __name__ == "__main__":
    import time
    rng = np.random.default_rng(0)
    x = rng.standard_normal((4, S, HIDDEN), dtype=np.float32)
    w1 = 0.02 * rng.standard_normal((2, NUM_BLOCKS, CH, CH)).astype(np.float32)
    b1 = 0.02 * rng.standard_normal((2, NUM_BLOCKS, CH)).astype(np.float32)
    w2 = 0.02 * rng.standard_normal((2, NUM_BLOCKS, CH, CH)).astype(np.float32)
    b2 = 0.02 * rng.standard_normal((2, NUM_BLOCKS, CH)).astype(np.float32)
    y1 = kernel(x, w1, b1, w2, b2)
    t0 = time.time(); y1 = kernel(x, w1, b1, w2, b2); t1 = time.time()
    y2 = _numpy_fallback(x, w1, b1, w2, b2)
    print("self-check rel err:",
          np.linalg.norm(y1 - y2) / np.linalg.norm(y2),
          "steady wall:", (t1 - t0) * 1e3, "ms")


# revision 8
# speedup vs baseline: 4.0364x; 2.0300x over previous
"""AFNO1D mixing kernel for 8 Trainium2 NeuronCores.

Strategy: tensor-parallel over the 8 block-diagonal channel blocks
(NUM_BLOCKS=8 == n_cores), one block of 96 channels per core, all 4
batch samples on every core. The rfft/irfft are factored into two-step
Cooley-Tukey real matmul stages (S=8192 = 64*128, modes k = 64e+d),
so the whole pipeline is real einsums + elementwise ops -- no FFT op,
which the Neuron compiler cannot ingest. Math validated to 3e-8 vs the
jax reference in float64 prototype.

  t = 128a + b:  G[d,b]   = sum_a cos/sin(2pi a d/64) x[128a+b] /sqrt(S)
  k = 64e + d:   X[64e+d] = sum_b w8192^{b(64e+d)} G[d,b]
  per-mode complex block MLP (relu, softshrink)
  inverse: A'[d,v] = sum_e ctil_k w8192^{+v k} O2[64e+d];
           y[128u+v] = Re sum_d w64^{+ud} A'[d,v];  y += x
"""
import numpy as np

HIDDEN = 768
NUM_BLOCKS = 8
CH = HIDDEN // NUM_BLOCKS  # 96
LAMBDA = 0.01
S, A_, B_, D_, E_ = 8192, 64, 128, 64, 65

_cache = {}


def _build_consts():
    a = np.arange(A_)[:, None]
    d = np.arange(D_)[None, :]
    ang = 2 * np.pi * a * d / 64.0
    st1c = (np.cos(ang) / np.sqrt(S)).astype(np.float32)   # [a, d]
    st1s = (-np.sin(ang) / np.sqrt(S)).astype(np.float32)  # [a, d]

    b = np.arange(B_)[:, None]
    e = np.arange(E_)[None, :]
    # M2[d]: [b, e] cos/sin of 2pi b (64e+d)/S
    ang2 = 2 * np.pi * b[None] * (64 * e[None] + np.arange(D_)[:, None, None]) / S
    M2r = np.cos(ang2).astype(np.float32)   # [d, b, e]
    M2i = (-np.sin(ang2)).astype(np.float32)
    M2r[1:, :, 64] = 0.0
    M2i[1:, :, 64] = 0.0

    ee = np.arange(E_)[:, None]
    v = np.arange(B_)[None, :]
    k3 = 64 * ee[None] + np.arange(D_)[:, None, None]       # [d, e, 1]
    ctil = np.where((k3 == 0) | (k3 == 4096), 1.0, 2.0) / np.sqrt(S)
    ang3 = 2 * np.pi * v[None] * k3 / S
    C3r = (ctil * np.cos(ang3)).astype(np.float32)          # [d, e, v]
    C3i = (ctil * np.sin(ang3)).astype(np.float32)
    C3r[1:, 64, :] = 0.0
    C3i[1:, 64, :] = 0.0

    u = np.arange(A_)[None, :]
    dc = np.arange(D_)[:, None]
    ST3c = np.cos(2 * np.pi * dc * u / 64.0).astype(np.float32)   # [d, u]
    ST3s = (-np.sin(2 * np.pi * dc * u / 64.0)).astype(np.float32)
    return st1c, st1s, M2r, M2i, C3r, C3i, ST3c, ST3s


def _make_block_fn(jnp):
    st1c, st1s, M2r, M2i, C3r, C3i, ST3c, ST3s = [
        jnp.asarray(c) for c in _build_consts()]

    def block_fn(xs, w1r, w1i, b1r, b1i, w2r, w2i, b2r, b2i):
        # xs: [4, 8192, 96] one channel block, all batches (bf16 on the wire)
        NB4 = xs.shape[0]
        xs = xs.astype(jnp.float32)
        xv = xs.reshape(NB4, A_, B_, CH)
        # stage 1 (contract a): G [n, d, b, c]
        Gr = jnp.einsum('ad,nabc->ndbc', st1c, xv)
        Gi = jnp.einsum('ad,nabc->ndbc', st1s, xv)
        # stage 2 (contract b): X [n, d, e, c]
        Xr = jnp.einsum('dbe,ndbc->ndec', M2r, Gr) - jnp.einsum('dbe,ndbc->ndec', M2i, Gi)
        Xi = jnp.einsum('dbe,ndbc->ndec', M2r, Gi) + jnp.einsum('dbe,ndbc->ndec', M2i, Gr)
        # MLP layer 1 (contract c)
        Hr = jnp.maximum(jnp.einsum('ndec,cj->ndej', Xr, w1r)
                         - jnp.einsum('ndec,cj->ndej', Xi, w1i) + b1r, 0.0)
        Hi = jnp.maximum(jnp.einsum('ndec,cj->ndej', Xi, w1r)
                         + jnp.einsum('ndec,cj->ndej', Xr, w1i) + b1i, 0.0)
        # MLP layer 2
        Or = (jnp.einsum('ndej,jc->ndec', Hr, w2r)
              - jnp.einsum('ndej,jc->ndec', Hi, w2i) + b2r)
        Oi = (jnp.einsum('ndej,jc->ndec', Hr, w2i)
              + jnp.einsum('ndej,jc->ndec', Hi, w2r) + b2i)
        Or = Or - jnp.clip(Or, -LAMBDA, LAMBDA)
        Oi = Oi - jnp.clip(Oi, -LAMBDA, LAMBDA)
        # inverse stage 1 (contract e): A' [n, d, v, c]
        Ar = jnp.einsum('dev,ndec->ndvc', C3r, Or) - jnp.einsum('dev,ndec->ndvc', C3i, Oi)
        Ai = jnp.einsum('dev,ndec->ndvc', C3i, Or) + jnp.einsum('dev,ndec->ndvc', C3r, Oi)
        # inverse stage 2 (contract d): y [n, u, v, c]
        y = jnp.einsum('du,ndvc->nuvc', ST3c, Ar) + jnp.einsum('du,ndvc->nuvc', ST3s, Ai)
        # return only the mixing term (|mix| ~ 0.05|y|); residual is added on
        # the host in fp32, so fp8 wire quantization stays ~5e-3 of |y|
        return y.reshape(NB4, S, CH).astype(jnp.float8_e5m2)

    return block_fn


def _get_impl():
    if 'impl' in _cache:
        return _cache['impl']
    impl = None
    try:
        import jax
        import jax.numpy as jnp
        devs = jax.devices()
        block_fn = _make_block_fn(jnp)
        if len(devs) >= NUM_BLOCKS:
            pfn = jax.pmap(block_fn, devices=devs[:NUM_BLOCKS])
            impl = ('pmap', pfn)
        else:
            impl = ('jit', jax.jit(jax.vmap(block_fn)))
    except Exception:
        impl = ('numpy', None)
    _cache['impl'] = impl
    return impl


def _numpy_fallback(x, w1, b1, w2, b2):
    xf = x.astype(np.float32)
    B = x.shape[0]
    X = np.fft.rfft(xf, axis=1, norm="ortho")
    X = X.reshape(B, -1, NUM_BLOCKS, CH)
    xr, xi = X.real.astype(np.float32), X.imag.astype(np.float32)
    mul = lambda inp, w: np.einsum("bmnd,ndk->bmnk", inp, w, optimize=True)
    o1r = np.maximum(mul(xr, w1[0]) - mul(xi, w1[1]) + b1[0], 0.0)
    o1i = np.maximum(mul(xi, w1[0]) + mul(xr, w1[1]) + b1[1], 0.0)
    o2r = mul(o1r, w2[0]) - mul(o1i, w2[1]) + b2[0]
    o2i = mul(o1r, w2[1]) + mul(o1i, w2[0]) + b2[1]
    o2r -= np.clip(o2r, -LAMBDA, LAMBDA)
    o2i -= np.clip(o2i, -LAMBDA, LAMBDA)
    Y = (o2r + 1j * o2i).reshape(B, -1, HIDDEN)
    y = np.fft.irfft(Y, n=S, axis=1, norm="ortho").astype(x.dtype)
    return y + x


def kernel(x, w1, b1, w2, b2):
    x = np.ascontiguousarray(np.asarray(x, dtype=np.float32))
    w1 = np.asarray(w1, np.float32); b1 = np.asarray(b1, np.float32)
    w2 = np.asarray(w2, np.float32); b2 = np.asarray(b2, np.float32)
    kind, fn = _get_impl()
    if kind == 'numpy':
        return _numpy_fallback(x, w1, b1, w2, b2)
    try:
        import time as _time
        import ml_dtypes
        B = x.shape[0]
        # [8, 4, 8192, 96] one channel block per device; fp8 e5m2 on the wire
        # both ways (mix-only output + host fp32 residual keeps total error
        # ~5e-3, well inside the 2e-2 tolerance)
        xsh = np.ascontiguousarray(
            x.reshape(B, S, NUM_BLOCKS, CH).transpose(2, 0, 1, 3)
        ).astype(ml_dtypes.float8_e5m2)
        args = (xsh, w1[0], w1[1], b1[0], b1[1], w2[0], w2[1], b2[0], b2[1])
        t0 = _time.time()
        out = fn(*args)
        out = np.asarray(out)  # [8, 4, 8192, 96] fp8 mix term
        globals()['_last_exec_ns'] = (_time.time() - t0) * 1e9
        mix = np.ascontiguousarray(
            out.astype(np.float32).transpose(1, 2, 0, 3).reshape(B, S, HIDDEN))
        return mix + x
    except Exception:
        _cache['impl'] = ('numpy', None)
        return _numpy_fallback(x, w1, b1, w2, b2)


if __name__ == "__main__":
    import time
    rng = np.random.default_rng(0)
    x = rng.standard_normal((4, S, HIDDEN), dtype=np.float32)
    w1 = 0.02 * rng.standard_normal((2, NUM_BLOCKS, CH, CH)).astype(np.float32)
    b1 = 0.02 * rng.standard_normal((2, NUM_BLOCKS, CH)).astype(np.float32)
    w2 = 0.02 * rng.standard_normal((2, NUM_BLOCKS, CH, CH)).astype(np.float32)
    b2 = 0.02 * rng.standard_normal((2, NUM_BLOCKS, CH)).astype(np.float32)
    y1 = kernel(x, w1, b1, w2, b2)
    t0 = time.time(); y1 = kernel(x, w1, b1, w2, b2); t1 = time.time()
    y2 = _numpy_fallback(x, w1, b1, w2, b2)
    print("self-check rel err:",
          np.linalg.norm(y1 - y2) / np.linalg.norm(y2),
          "steady wall:", (t1 - t0) * 1e3, "ms")


# revision 10
# speedup vs baseline: 4.1866x; 1.0372x over previous
"""AFNO1D mixing kernel for 8 Trainium2 NeuronCores.

Strategy: tensor-parallel over the 8 block-diagonal channel blocks
(NUM_BLOCKS=8 == n_cores), one block of 96 channels per core, all 4
batch samples on every core. The rfft/irfft are factored into two-step
Cooley-Tukey real matmul stages (S=8192 = 64*128, modes k = 64e+d),
so the whole pipeline is real einsums + elementwise ops -- no FFT op,
which the Neuron compiler cannot ingest. Math validated to 3e-8 vs the
jax reference in float64 prototype.

  t = 128a + b:  G[d,b]   = sum_a cos/sin(2pi a d/64) x[128a+b] /sqrt(S)
  k = 64e + d:   X[64e+d] = sum_b w8192^{b(64e+d)} G[d,b]
  per-mode complex block MLP (relu, softshrink)
  inverse: A'[d,v] = sum_e ctil_k w8192^{+v k} O2[64e+d];
           y[128u+v] = Re sum_d w64^{+ud} A'[d,v];  y += x
"""
import numpy as np

HIDDEN = 768
NUM_BLOCKS = 8
CH = HIDDEN // NUM_BLOCKS  # 96
LAMBDA = 0.01
S, A_, B_, D_, E_ = 8192, 64, 128, 64, 65

_cache = {}


def _build_consts():
    a = np.arange(A_)[:, None]
    d = np.arange(D_)[None, :]
    ang = 2 * np.pi * a * d / 64.0
    st1c = (np.cos(ang) / np.sqrt(S)).astype(np.float32)   # [a, d]
    st1s = (-np.sin(ang) / np.sqrt(S)).astype(np.float32)  # [a, d]

    b = np.arange(B_)[:, None]
    e = np.arange(E_)[None, :]
    # M2[d]: [b, e] cos/sin of 2pi b (64e+d)/S
    ang2 = 2 * np.pi * b[None] * (64 * e[None] + np.arange(D_)[:, None, None]) / S
    M2r = np.cos(ang2).astype(np.float32)   # [d, b, e]
    M2i = (-np.sin(ang2)).astype(np.float32)
    M2r[1:, :, 64] = 0.0
    M2i[1:, :, 64] = 0.0

    ee = np.arange(E_)[:, None]
    v = np.arange(B_)[None, :]
    k3 = 64 * ee[None] + np.arange(D_)[:, None, None]       # [d, e, 1]
    ctil = np.where((k3 == 0) | (k3 == 4096), 1.0, 2.0) / np.sqrt(S)
    ang3 = 2 * np.pi * v[None] * k3 / S
    C3r = (ctil * np.cos(ang3)).astype(np.float32)          # [d, e, v]
    C3i = (ctil * np.sin(ang3)).astype(np.float32)
    C3r[1:, 64, :] = 0.0
    C3i[1:, 64, :] = 0.0

    u = np.arange(A_)[None, :]
    dc = np.arange(D_)[:, None]
    ST3c = np.cos(2 * np.pi * dc * u / 64.0).astype(np.float32)   # [d, u]
    ST3s = (-np.sin(2 * np.pi * dc * u / 64.0)).astype(np.float32)
    return st1c, st1s, M2r, M2i, C3r, C3i, ST3c, ST3s


def _make_block_fn(jnp):
    st1c, st1s, M2r, M2i, C3r, C3i, ST3c, ST3s = [
        jnp.asarray(c) for c in _build_consts()]

    def block_fn(xs, w1r, w1i, b1r, b1i, w2r, w2i, b2r, b2i):
        # xs: [4, 8192, 96] one channel block, all batches (bf16 on the wire)
        NB4 = xs.shape[0]
        xs = xs.astype(jnp.float32)
        xv = xs.reshape(NB4, A_, B_, CH)
        # stage 1 (contract a): G [n, d, b, c]
        Gr = jnp.einsum('ad,nabc->ndbc', st1c, xv)
        Gi = jnp.einsum('ad,nabc->ndbc', st1s, xv)
        # stage 2 (contract b): X [n, d, e, c]
        Xr = jnp.einsum('dbe,ndbc->ndec', M2r, Gr) - jnp.einsum('dbe,ndbc->ndec', M2i, Gi)
        Xi = jnp.einsum('dbe,ndbc->ndec', M2r, Gi) + jnp.einsum('dbe,ndbc->ndec', M2i, Gr)
        # MLP layer 1 (contract c)
        Hr = jnp.maximum(jnp.einsum('ndec,cj->ndej', Xr, w1r)
                         - jnp.einsum('ndec,cj->ndej', Xi, w1i) + b1r, 0.0)
        Hi = jnp.maximum(jnp.einsum('ndec,cj->ndej', Xi, w1r)
                         + jnp.einsum('ndec,cj->ndej', Xr, w1i) + b1i, 0.0)
        # MLP layer 2
        Or = (jnp.einsum('ndej,jc->ndec', Hr, w2r)
              - jnp.einsum('ndej,jc->ndec', Hi, w2i) + b2r)
        Oi = (jnp.einsum('ndej,jc->ndec', Hr, w2i)
              + jnp.einsum('ndej,jc->ndec', Hi, w2r) + b2i)
        Or = Or - jnp.clip(Or, -LAMBDA, LAMBDA)
        Oi = Oi - jnp.clip(Oi, -LAMBDA, LAMBDA)
        # inverse stage 1 (contract e): A' [n, d, v, c]
        Ar = jnp.einsum('dev,ndec->ndvc', C3r, Or) - jnp.einsum('dev,ndec->ndvc', C3i, Oi)
        Ai = jnp.einsum('dev,ndec->ndvc', C3i, Or) + jnp.einsum('dev,ndec->ndvc', C3r, Oi)
        # inverse stage 2 (contract d): y [n, u, v, c]
        y = jnp.einsum('du,ndvc->nuvc', ST3c, Ar) + jnp.einsum('du,ndvc->nuvc', ST3s, Ai)
        # return only the mixing term (|mix| ~ 0.05|y|); residual is added on
        # the host in fp32, so fp8 wire quantization stays ~5e-3 of |y|
        return y.reshape(NB4, S, CH).astype(jnp.float8_e5m2)

    return block_fn


def _get_impl():
    if 'impl' in _cache:
        return _cache['impl']
    impl = None
    try:
        import jax
        import jax.numpy as jnp
        devs = jax.devices()
        block_fn = _make_block_fn(jnp)
        if len(devs) >= NUM_BLOCKS:
            pfn = jax.pmap(block_fn, devices=devs[:NUM_BLOCKS])
            impl = ('pmap', pfn)
        else:
            impl = ('jit', jax.jit(jax.vmap(block_fn)))
    except Exception:
        impl = ('numpy', None)
    _cache['impl'] = impl
    return impl


def _numpy_fallback(x, w1, b1, w2, b2):
    xf = x.astype(np.float32)
    B = x.shape[0]
    X = np.fft.rfft(xf, axis=1, norm="ortho")
    X = X.reshape(B, -1, NUM_BLOCKS, CH)
    xr, xi = X.real.astype(np.float32), X.imag.astype(np.float32)
    mul = lambda inp, w: np.einsum("bmnd,ndk->bmnk", inp, w, optimize=True)
    o1r = np.maximum(mul(xr, w1[0]) - mul(xi, w1[1]) + b1[0], 0.0)
    o1i = np.maximum(mul(xi, w1[0]) + mul(xr, w1[1]) + b1[1], 0.0)
    o2r = mul(o1r, w2[0]) - mul(o1i, w2[1]) + b2[0]
    o2i = mul(o1r, w2[1]) + mul(o1i, w2[0]) + b2[1]
    o2r -= np.clip(o2r, -LAMBDA, LAMBDA)
    o2i -= np.clip(o2i, -LAMBDA, LAMBDA)
    Y = (o2r + 1j * o2i).reshape(B, -1, HIDDEN)
    y = np.fft.irfft(Y, n=S, axis=1, norm="ortho").astype(x.dtype)
    return y + x


def kernel(x, w1, b1, w2, b2):
    x = np.ascontiguousarray(np.asarray(x, dtype=np.float32))
    w1 = np.asarray(w1, np.float32); b1 = np.asarray(b1, np.float32)
    w2 = np.asarray(w2, np.float32); b2 = np.asarray(b2, np.float32)
    kind, fn = _get_impl()
    if kind == 'numpy':
        return _numpy_fallback(x, w1, b1, w2, b2)
    try:
        import time as _time
        import ml_dtypes
        B = x.shape[0]
        # [8, 4, 8192, 96] one channel block per device; fp8 e5m2 on the wire
        # both ways (mix-only output + host fp32 residual keeps total error
        # ~5e-3, well inside the 2e-2 tolerance). Cast to fp8 BEFORE the
        # host transpose so the shuffle moves 25MB, not 100MB.
        x8 = x.reshape(B, S, NUM_BLOCKS, CH).astype(ml_dtypes.float8_e5m2)
        xsh = np.ascontiguousarray(x8.transpose(2, 0, 1, 3))
        args = (xsh, w1[0], w1[1], b1[0], b1[1], w2[0], w2[1], b2[0], b2[1])
        t0 = _time.time()
        out = fn(*args)
        out = np.asarray(out)  # [8, 4, 8192, 96] fp8 mix term
        globals()['_last_exec_ns'] = (_time.time() - t0) * 1e9
        # transpose while still fp8 (25MB), then upcast and add in one pass.
        # e5m2 -> f32 via the fp16 bit layout (e5m2 == fp16's top byte),
        # ~2x faster than ml_dtypes astype.
        mix8 = np.ascontiguousarray(out.transpose(1, 2, 0, 3)).reshape(B, S, HIDDEN)
        res = (mix8.view(np.uint8).astype(np.uint16) << np.uint16(8)).view(
            np.float16).astype(np.float32)
        np.add(res, x, out=res)
        return res
    except Exception:
        _cache['impl'] = ('numpy', None)
        return _numpy_fallback(x, w1, b1, w2, b2)


if __name__ == "__main__":
    import time
    rng = np.random.default_rng(0)
    x = rng.standard_normal((4, S, HIDDEN), dtype=np.float32)
    w1 = 0.02 * rng.standard_normal((2, NUM_BLOCKS, CH, CH)).astype(np.float32)
    b1 = 0.02 * rng.standard_normal((2, NUM_BLOCKS, CH)).astype(np.float32)
    w2 = 0.02 * rng.standard_normal((2, NUM_BLOCKS, CH, CH)).astype(np.float32)
    b2 = 0.02 * rng.standard_normal((2, NUM_BLOCKS, CH)).astype(np.float32)
    y1 = kernel(x, w1, b1, w2, b2)
    t0 = time.time(); y1 = kernel(x, w1, b1, w2, b2); t1 = time.time()
    y2 = _numpy_fallback(x, w1, b1, w2, b2)
    print("self-check rel err:",
          np.linalg.norm(y1 - y2) / np.linalg.norm(y2),
          "steady wall:", (t1 - t0) * 1e3, "ms")
